# revision 10
# baseline (speedup 1.0000x reference)
"""Trainium2 Bass kernel for nn_CNNTeacherModel_14551349198856 (moe_routing).

Reference computation: for each row i of hidden_state [8192, 1024]:
    out[i] = W[group[i]] @ hidden[i] + b[group[i]]   if group[i] < 5
    out[i] = float(labels[i])  (broadcast over L)    if group[i] == 5

Strategy (MoE routing — compute only the selected head per row, 5x fewer
FLOPs than the reference's all-heads einsum):
  * Host: sort active rows (group<5) by group, deal them round-robin to 4
    batch shards so every shard has identical per-group row counts (pad to
    a 128 multiple per group with dummy rows).  The L=1024 output dim is
    split in 2.  Core (s, l) of the 4x2 grid computes its shard's rows for
    L-half l.
  * Device (per core): x and W live in SBUF, loaded with one big DMA per
    (tensor, group) in a host-packed [128, KT*cols] layout (6-8KB lines,
    few issue instructions — HWDGE issue costs ~0.6us each).  For each
    128-row M-tile (statically known group): 8 accumulating matmuls over
    the contraction (H) into one PSUM bank, plus a 9th K=1 matmul
    (ones[128] x bias_row) adding the bias, then a VectorE eviction into a
    per-segment output tile and one store DMA per segment.
  * Transport dtype is bf16 (x, W, bias, y) to halve HBM traffic — the
    kernel is HBM-bound (~275 GB/s/core).  PSUM accumulates in fp32.
    Error vs the fp32 reference is ~1.3e-2 absolute on logits of scale ~3,
    i.e. ~1.3e-5 of the output absmax (label rows dominate at 1023).
    Set MOE_FP32R=1 for the fp32r path (~5e-4 absolute) at 2x DMA bytes.
  * A warmup chain of matmuls lifts the PE HAM clock-gate to 2.4 GHz
    while the first loads stream.
  * Host: scatter device outputs back by the inverse permutation; fill
    group==5 rows from labels.
"""

import math
import os

import numpy as np

B, H, L, NH = 8192, 1024, 1024, 5
PB, PL = 4, 2          # batch shards x L shards = 8 cores
LS = L // PL           # 512 output columns per core
KT = H // 128          # 8 contraction tiles
N_CORES = PB * PL
N_WARMUP = int(os.environ.get("MOE_WARMUP", "32"))

USE_FP32R = bool(int(os.environ.get("MOE_FP32R", "0")))

# stash of the last BassKernelResults (so a test harness can read
# exec_time_ns when tracing is enabled via BASS_TRACE)
LAST_RESULTS = None


def _split_excess_waits(nc, mybir, cap=1):
    """Walrus in this toolchain rejects >cap embedded sync-waits per
    instruction ("Too many sync wait commands").  Hoist excess waits into
    fresh same-engine InstNoOps placed immediately before the instruction
    (sequencers execute waits in stream order, so semantics are identical)."""
    for f in nc.m.functions:
        for blk in f.blocks:
            insts = list(blk.instructions)
            new = []
            changed = False
            for inst in insts:
                try:
                    si = inst.sync_info
                except AttributeError:
                    si = None
                waits = list(si.on_wait) if si else []
                if len(waits) > cap:
                    changed = True
                    excess, keep = waits[:-cap], waits[-cap:]
                    for i in range(0, len(excess), cap):
                        new.append(
                            mybir.InstNoOp(
                                name=nc.get_next_instruction_name(),
                                sync_info=mybir.SyncInfo(
                                    on_wait=excess[i:i + cap], on_update=[]
                                ),
                                bass_nofuse=True,
                                engine=inst.engine,
                            )
                        )
                    inst.sync_info = mybir.SyncInfo(
                        on_wait=keep, on_update=list(si.on_update)
                    )
                new.append(inst)
            if changed:
                blk.instructions = new


def _build_program(n_seg):
    """Build the per-core Bass program.  n_seg[g] = rows (multiple of 128)
    this core computes for group g; R = sum(n_seg).

    DRAM layouts (host-packed):
      xp  [128, KT*R]     xp[p, off_g*KT + h*n_g + r] = x_row[idx[g,r]][h*128+p]
      wp  [128, NH*KT*LS] wp[p, (g*KT+h)*LS + j]      = W[g][l0+j, h*128+p]
      bp  [1, NH*LS]      bp[0, g*LS + j]             = b[g, l0+j]
      y   [128, T*LS]     y[p, t*LS + j] = out row (t*128+p) col j   (T tiles)
    """
    import concourse.bass as bass
    import concourse.mybir as mybir
    import concourse.tile as tile

    R = sum(n_seg)
    T = R // 128
    f32 = mybir.dt.float32
    mm_dt = mybir.dt.float32r if USE_FP32R else mybir.dt.bfloat16
    io_dt = mybir.dt.float32 if USE_FP32R else mybir.dt.bfloat16

    nc = bass.Bass()
    xdr = nc.dram_tensor("xp", [128, KT * R], mm_dt, kind="ExternalInput")
    wdr = nc.dram_tensor("wp", [128, NH * KT * LS], mm_dt, kind="ExternalInput")
    bdr = nc.dram_tensor("bp", [1, NH * LS], mm_dt, kind="ExternalInput")
    y = nc.dram_tensor("y", [128, T * LS], io_dt, kind="ExternalOutput")

    with tile.TileContext(nc) as tc:
        with (
            tc.tile_pool(name="xp_sb", bufs=1) as xp_sb,
            tc.tile_pool(name="wp_sb", bufs=1) as wp_sb,
            tc.tile_pool(name="cp", bufs=1) as cp,
            tc.tile_pool(name="pp", bufs=4, space="PSUM") as pp,
            tc.tile_pool(name="wup", bufs=1, space="PSUM") as wup,
            tc.tile_pool(name="op", bufs=2) as op,
        ):
            # --- PE warmup: keep the HAM clock-gate open while DMAs stream.
            # The psum bank is never read.
            wu_x = cp.tile([128, 128], mm_dt, tag="wux", name="wux")
            wu_w = cp.tile([128, LS], mm_dt, tag="wuw", name="wuw")
            nc.gpsimd.memset(wu_x[:], 0.0)
            nc.gpsimd.memset(wu_w[:], 0.0)
            wu_ps = wup.tile([128, LS], f32, name="wups")
            for _ in range(N_WARMUP):
                nc.tensor.matmul(wu_ps[:], wu_x[:], wu_w[:], start=True, stop=True)

            # --- ones row for the K=1 bias matmul; bias rows tile
            ones_t = cp.tile([1, 128], mm_dt, tag="ones", name="ones")
            nc.vector.memset(ones_t[:], 1.0)
            bias_t = cp.tile([1, NH * LS], mm_dt, tag="bias", name="bias")
            nc.scalar.dma_start(out=bias_t[:], in_=bdr[:])

            # two HWDGE queues (SP + ACT); alternate the big loads
            ld_engines = [nc.sync, nc.scalar]

            xts, wts = [], []
            xoff = 0
            for g in range(NH):
                ng = n_seg[g]
                if ng == 0:
                    xts.append(None)
                    wts.append(None)
                    continue
                # alternate queues per segment so both HWDGE queues carry
                # ~half the bytes and segment g's two loads finish together
                xt_t = xp_sb.tile([128, KT * ng], mm_dt, tag=f"x{g}", name=f"x{g}")
                ld_engines[g % 2].dma_start(
                    out=xt_t[:], in_=xdr[:, xoff:xoff + KT * ng]
                )
                xts.append(xt_t)
                wt_t = wp_sb.tile([128, KT * LS], mm_dt, tag=f"w{g}", name=f"w{g}")
                ld_engines[(g + 1) % 2].dma_start(
                    out=wt_t[:],
                    in_=wdr[:, g * KT * LS:(g + 1) * KT * LS],
                )
                wts.append(wt_t)
                xoff += KT * ng

            tglob = 0
            for g in range(NH):
                ng = n_seg[g]
                if ng == 0:
                    continue
                nt = ng // 128
                for t in range(nt):
                    ps = pp.tile([128, LS], f32, tag="ps", name=f"ps{g}_{t}")
                    for h in range(KT):
                        nc.tensor.matmul(
                            ps[:],
                            xts[g][:, h * ng + t * 128:h * ng + (t + 1) * 128],
                            wts[g][:, h * LS:(h + 1) * LS],
                            start=(h == 0),
                            stop=False,
                        )
                    # bias: ones[1,128].T @ b_row[1,LS] accumulated on top
                    nc.tensor.matmul(
                        ps[:], ones_t[:], bias_t[0:1, g * LS:(g + 1) * LS],
                        start=False, stop=True,
                    )
                    ot = op.tile([128, LS], io_dt, tag="ot", name=f"ot{g}_{t}")
                    nc.vector.tensor_copy(ot[:], ps[:])
                    nc.sync.dma_start(
                        out=y[:, (tglob + t) * LS:(tglob + t + 1) * LS], in_=ot[:]
                    )
                tglob += nt

    _split_excess_waits(nc, mybir)
    return nc


def kernel(hidden_state, W, b, group, labels):
    global LAST_RESULTS
    import ml_dtypes
    from concourse.bass_utils import run_bass_kernel_spmd

    hidden_state = np.ascontiguousarray(np.asarray(hidden_state, dtype=np.float32))
    W = np.asarray(W, dtype=np.float32)
    b = np.asarray(b, dtype=np.float32)
    group = np.asarray(group)
    labels = np.asarray(labels)

    np_io = np.float32 if USE_FP32R else ml_dtypes.bfloat16

    g64 = group.astype(np.int64)
    active = np.nonzero(g64 < NH)[0]
    order = np.argsort(g64[active], kind="stable")
    sidx = active[order]
    counts = np.bincount(g64[active], minlength=NH)

    # per-shard rows per group, padded to a multiple of 128
    n_seg = []
    for g in range(NH):
        n = math.ceil(counts[g] / PB) if counts[g] else 0
        n_seg.append(128 * math.ceil(n / 128) if n else 0)
    R = sum(n_seg)
    T = R // 128

    # deal rows: shard s takes every PB-th row of each group's sorted run
    idx = np.full((PB, R), -1, dtype=np.int64)
    off = 0
    roff = 0
    for g in range(NH):
        rows = sidx[off:off + counts[g]]
        for s in range(PB):
            sub = rows[s::PB]
            idx[s, roff:roff + len(sub)] = sub
        off += counts[g]
        roff += n_seg[g]

    # pack x per shard: [128, KT*R], seg-major, h-blocks contiguous per seg
    xpacks = []
    for s in range(PB):
        xg = hidden_state[np.maximum(idx[s], 0)].astype(np_io)  # [R, H]
        parts = []
        roff = 0
        for g in range(NH):
            ng = n_seg[g]
            if ng == 0:
                continue
            seg = xg[roff:roff + ng]                       # [ng, H]
            seg = seg.reshape(ng, KT, 128).transpose(2, 1, 0)  # [128, KT, ng]
            parts.append(seg.reshape(128, KT * ng))
            roff += ng
        xpacks.append(np.ascontiguousarray(np.concatenate(parts, axis=1)))

    # pack W per L-half: [128, NH*KT*LS]; bias [1, NH*LS]
    wpacks = []
    bpacks = []
    for l in range(PL):
        parts = []
        for g in range(NH):
            wg = W[g].T[:, l * LS:(l + 1) * LS].astype(np_io)   # [H, LS]
            wg = wg.reshape(KT, 128, LS).transpose(1, 0, 2)     # [128, KT, LS]
            parts.append(wg.reshape(128, KT * LS))
        wpacks.append(np.ascontiguousarray(np.concatenate(parts, axis=1)))
        bpacks.append(
            np.ascontiguousarray(
                b[:, l * LS:(l + 1) * LS].astype(np_io).reshape(1, NH * LS)
            )
        )

    in_maps = []
    for c in range(N_CORES):
        s, l = divmod(c, PL)
        in_maps.append({"xp": xpacks[s], "wp": wpacks[l], "bp": bpacks[l]})

    nc = _build_program(n_seg)
    res = run_bass_kernel_spmd(nc, in_maps, list(range(N_CORES)))
    LAST_RESULTS = res

    out = np.empty((B, L), dtype=np.float32)
    lab_rows = g64 == NH
    out[lab_rows] = labels[lab_rows, None].astype(np.float32)
    for c in range(N_CORES):
        s, l = divmod(c, PL)
        yp = res.results[c]["y"].astype(np.float32)       # [128, T*LS]
        yg = yp.reshape(128, T, LS).transpose(1, 0, 2).reshape(R, LS)
        m = idx[s] >= 0
        out[idx[s][m], l * LS:(l + 1) * LS] = yg[m]
    return out


# revision 14
# speedup vs baseline: 1.1079x; 1.1079x over previous
"""Trainium2 Bass kernel for nn_CNNTeacherModel_14551349198856 (moe_routing).

Reference computation: for each row i of hidden_state [8192, 1024]:
    out[i] = W[group[i]] @ hidden[i] + b[group[i]]   if group[i] < 5
    out[i] = float(labels[i])  (broadcast over L)    if group[i] == 5

Strategy (MoE routing — compute only the selected head per row, 5x fewer
FLOPs than the reference's all-heads einsum):
  * Host: sort active rows (group<5) by group, deal them round-robin to 4
    batch shards so every shard has identical per-group row counts (pad to
    a 128 multiple per group with dummy rows).  The L=1024 output dim is
    split in 2.  Core (s, l) of the 4x2 grid computes its shard's rows for
    L-half l.
  * Device (per core): x and W live in SBUF, loaded with one big DMA per
    (tensor, group) in a host-packed [128, KT*cols] layout (6-8KB lines,
    few issue instructions — HWDGE issue costs ~0.6us each).  For each
    128-row M-tile (statically known group): 8 accumulating matmuls over
    the contraction (H) into one PSUM bank, plus a 9th K=1 matmul
    (ones[128] x bias_row) adding the bias, then a VectorE eviction into a
    per-segment output tile and one store DMA per segment.
  * Transport dtype is bf16 (x, W, bias, y) to halve HBM traffic — the
    kernel is HBM-bound (~275 GB/s/core).  PSUM accumulates in fp32.
    Error vs the fp32 reference is ~1.3e-2 absolute on logits of scale ~3,
    i.e. ~1.3e-5 of the output absmax (label rows dominate at 1023).
    Set MOE_FP32R=1 for the fp32r path (~5e-4 absolute) at 2x DMA bytes.
  * A warmup chain of matmuls lifts the PE HAM clock-gate to 2.4 GHz
    while the first loads stream.
  * Host: scatter device outputs back by the inverse permutation; fill
    group==5 rows from labels.
"""

import math
import os

import numpy as np

B, H, L, NH = 8192, 1024, 1024, 5
PB, PL = 4, 2          # batch shards x L shards = 8 cores
LS = L // PL           # 512 output columns per core
KT = H // 128          # 8 contraction tiles
N_CORES = PB * PL
N_WARMUP = int(os.environ.get("MOE_WARMUP", "32"))

USE_FP32R = bool(int(os.environ.get("MOE_FP32R", "0")))

# stash of the last BassKernelResults (so a test harness can read
# exec_time_ns when tracing is enabled via BASS_TRACE)
LAST_RESULTS = None


def _split_excess_waits(nc, mybir, cap=1):
    """Walrus in this toolchain rejects >cap embedded sync-waits per
    instruction ("Too many sync wait commands").  Hoist excess waits into
    fresh same-engine InstNoOps placed immediately before the instruction
    (sequencers execute waits in stream order, so semantics are identical)."""
    for f in nc.m.functions:
        for blk in f.blocks:
            insts = list(blk.instructions)
            new = []
            changed = False
            for inst in insts:
                try:
                    si = inst.sync_info
                except AttributeError:
                    si = None
                waits = list(si.on_wait) if si else []
                if len(waits) > cap:
                    changed = True
                    excess, keep = waits[:-cap], waits[-cap:]
                    for i in range(0, len(excess), cap):
                        new.append(
                            mybir.InstNoOp(
                                name=nc.get_next_instruction_name(),
                                sync_info=mybir.SyncInfo(
                                    on_wait=excess[i:i + cap], on_update=[]
                                ),
                                bass_nofuse=True,
                                engine=inst.engine,
                            )
                        )
                    inst.sync_info = mybir.SyncInfo(
                        on_wait=keep, on_update=list(si.on_update)
                    )
                new.append(inst)
            if changed:
                blk.instructions = new


def _build_program(n_seg):
    """Build the per-core Bass program.  n_seg[g] = rows (multiple of 128)
    this core computes for group g; R = sum(n_seg).

    DRAM layouts (host-packed):
      xp  [128, T*KT*128] xp[p, (t*KT+h)*128 + r] = x_row[t*128+r][h*128+p]
                          (tile-major so each M-tile is one contiguous load)
      wp  [128, NH*KT*LS] wp[p, (g*KT+h)*LS + j]  = W[g][l0+j, h*128+p]
      bp  [1, NH*LS]      bp[0, g*LS + j]         = b[g, l0+j]
      y   [128, T*LS]     y[p, t*LS + j] = out row (t*128+p) col j   (T tiles)
    """
    import concourse.bass as bass
    import concourse.mybir as mybir
    import concourse.tile as tile

    R = sum(n_seg)
    T = R // 128
    f32 = mybir.dt.float32
    mm_dt = mybir.dt.float32r if USE_FP32R else mybir.dt.bfloat16
    io_dt = mybir.dt.float32 if USE_FP32R else mybir.dt.bfloat16

    nc = bass.Bass()
    xdr = nc.dram_tensor("xp", [128, KT * R], mm_dt, kind="ExternalInput")
    wdr = nc.dram_tensor("wp", [128, NH * KT * LS], mm_dt, kind="ExternalInput")
    bdr = nc.dram_tensor("bp", [1, NH * LS], mm_dt, kind="ExternalInput")
    y = nc.dram_tensor("y", [128, T * LS], io_dt, kind="ExternalOutput")

    with tile.TileContext(nc) as tc:
        with (
            tc.tile_pool(name="xp_sb", bufs=1) as xp_sb,
            tc.tile_pool(name="wp_sb", bufs=1) as wp_sb,
            tc.tile_pool(name="cp", bufs=1) as cp,
            tc.tile_pool(name="pp", bufs=4, space="PSUM") as pp,
            tc.tile_pool(name="wup", bufs=1, space="PSUM") as wup,
            tc.tile_pool(name="op", bufs=2) as op,
        ):
            # --- PE warmup: keep the HAM clock-gate open while DMAs stream.
            # The psum bank is never read.
            wu_x = cp.tile([128, 128], mm_dt, tag="wux", name="wux")
            wu_w = cp.tile([128, LS], mm_dt, tag="wuw", name="wuw")
            nc.gpsimd.memset(wu_x[:], 0.0)
            nc.gpsimd.memset(wu_w[:], 0.0)
            wu_ps = wup.tile([128, LS], f32, name="wups")
            for _ in range(N_WARMUP):
                nc.tensor.matmul(wu_ps[:], wu_x[:], wu_w[:], start=True, stop=True)

            # --- ones row for the K=1 bias matmul; bias rows tile
            ones_t = cp.tile([1, 128], mm_dt, tag="ones", name="ones")
            nc.vector.memset(ones_t[:], 1.0)
            bias_t = cp.tile([1, NH * LS], mm_dt, tag="bias", name="bias")
            nc.scalar.dma_start(out=bias_t[:], in_=bdr[:])

            # two HWDGE queues (SP + ACT); alternate the big loads
            ld_engines = [nc.sync, nc.scalar]

            # x loads: one contiguous DMA per M-tile (tile-major packing);
            # W loads: one DMA per group.  Issue in consumption order,
            # alternating the two HWDGE queues.
            TKT = KT * 128
            wts = []
            xtiles = []
            ld_i = 0
            tglob = 0
            for g in range(NH):
                ng = n_seg[g]
                if ng == 0:
                    wts.append(None)
                    continue
                wt_t = wp_sb.tile([128, KT * LS], mm_dt, tag=f"w{g}", name=f"w{g}")
                ld_engines[ld_i % 2].dma_start(
                    out=wt_t[:],
                    in_=wdr[:, g * KT * LS:(g + 1) * KT * LS],
                )
                ld_i += 1
                wts.append(wt_t)
                for t in range(ng // 128):
                    xt_t = xp_sb.tile([128, TKT], mm_dt, tag=f"xt{tglob}",
                                      name=f"xt{tglob}")
                    ld_engines[ld_i % 2].dma_start(
                        out=xt_t[:],
                        in_=xdr[:, tglob * TKT:(tglob + 1) * TKT],
                    )
                    ld_i += 1
                    xtiles.append(xt_t)
                    tglob += 1

            tglob = 0
            for g in range(NH):
                ng = n_seg[g]
                if ng == 0:
                    continue
                nt = ng // 128
                ot = op.tile([128, nt * LS], io_dt, tag="ot", name=f"ot{g}")
                for t in range(nt):
                    ps = pp.tile([128, LS], f32, tag="ps", name=f"ps{g}_{t}")
                    xt_t = xtiles[tglob + t]
                    for h in range(KT):
                        nc.tensor.matmul(
                            ps[:],
                            xt_t[:, h * 128:(h + 1) * 128],
                            wts[g][:, h * LS:(h + 1) * LS],
                            start=(h == 0),
                            stop=False,
                        )
                    # bias: ones[1,128].T @ b_row[1,LS] accumulated on top
                    nc.tensor.matmul(
                        ps[:], ones_t[:], bias_t[0:1, g * LS:(g + 1) * LS],
                        start=False, stop=True,
                    )
                    nc.vector.tensor_copy(ot[:, t * LS:(t + 1) * LS], ps[:])
                # one store per segment
                nc.sync.dma_start(
                    out=y[:, tglob * LS:(tglob + nt) * LS], in_=ot[:]
                )
                tglob += nt

    _split_excess_waits(nc, mybir)
    return nc


def kernel(hidden_state, W, b, group, labels):
    global LAST_RESULTS
    import ml_dtypes
    from concourse.bass_utils import run_bass_kernel_spmd

    hidden_state = np.ascontiguousarray(np.asarray(hidden_state, dtype=np.float32))
    W = np.asarray(W, dtype=np.float32)
    b = np.asarray(b, dtype=np.float32)
    group = np.asarray(group)
    labels = np.asarray(labels)

    np_io = np.float32 if USE_FP32R else ml_dtypes.bfloat16

    g64 = group.astype(np.int64)
    active = np.nonzero(g64 < NH)[0]
    order = np.argsort(g64[active], kind="stable")
    sidx = active[order]
    counts = np.bincount(g64[active], minlength=NH)

    # per-shard rows per group, padded to a multiple of 128
    n_seg = []
    for g in range(NH):
        n = math.ceil(counts[g] / PB) if counts[g] else 0
        n_seg.append(128 * math.ceil(n / 128) if n else 0)
    R = sum(n_seg)
    T = R // 128

    # deal rows: shard s takes every PB-th row of each group's sorted run
    idx = np.full((PB, R), -1, dtype=np.int64)
    off = 0
    roff = 0
    for g in range(NH):
        rows = sidx[off:off + counts[g]]
        for s in range(PB):
            sub = rows[s::PB]
            idx[s, roff:roff + len(sub)] = sub
        off += counts[g]
        roff += n_seg[g]

    # pack x per shard: [128, T*KT*128], M-tile-major so each tile is one
    # contiguous DMA: xp[p, (t*KT+h)*128 + r] = xg[t*128+r, h*128+p]
    xpacks = []
    for s in range(PB):
        xg = hidden_state[np.maximum(idx[s], 0)].astype(np_io)  # [R, H]
        xp = xg.reshape(T, 128, KT, 128).transpose(3, 0, 2, 1)  # [p, t, h, r]
        xpacks.append(np.ascontiguousarray(xp.reshape(128, T * KT * 128)))

    # pack W per L-half: [128, NH*KT*LS]; bias [1, NH*LS]
    wpacks = []
    bpacks = []
    for l in range(PL):
        parts = []
        for g in range(NH):
            wg = W[g].T[:, l * LS:(l + 1) * LS].astype(np_io)   # [H, LS]
            wg = wg.reshape(KT, 128, LS).transpose(1, 0, 2)     # [128, KT, LS]
            parts.append(wg.reshape(128, KT * LS))
        wpacks.append(np.ascontiguousarray(np.concatenate(parts, axis=1)))
        bpacks.append(
            np.ascontiguousarray(
                b[:, l * LS:(l + 1) * LS].astype(np_io).reshape(1, NH * LS)
            )
        )

    in_maps = []
    for c in range(N_CORES):
        s, l = divmod(c, PL)
        in_maps.append({"xp": xpacks[s], "wp": wpacks[l], "bp": bpacks[l]})

    nc = _build_program(n_seg)
    res = run_bass_kernel_spmd(nc, in_maps, list(range(N_CORES)))
    LAST_RESULTS = res

    out = np.empty((B, L), dtype=np.float32)
    lab_rows = g64 == NH
    out[lab_rows] = labels[lab_rows, None].astype(np.float32)
    for c in range(N_CORES):
        s, l = divmod(c, PL)
        yp = res.results[c]["y"].astype(np.float32)       # [128, T*LS]
        yg = yp.reshape(128, T, LS).transpose(1, 0, 2).reshape(R, LS)
        m = idx[s] >= 0
        out[idx[s][m], l * LS:(l + 1) * LS] = yg[m]
    return out
